# revision 42
# baseline (speedup 1.0000x reference)
"""GroupNorm + single-head self-attention block (B=16, C=512, H=W=32) on 8
TRN2 NeuronCores.

Sharding: pure data-parallel over batch - 2 samples per core, no collectives.

Fused-weight fp8 pipeline, paired-PSUM evacuations, batched GroupNorm.

Host constant-folding collapses the four C*C projections to two:

  M  = wq^T wk              scores = h^T M h     (q/k fused)
  W2 = wo wv                o2     = attn^T (W2 h)  (v/o fused)

Per-sample dataflow (C=512 channels, N=1024 pixels), channels/pixels on
partitions, every big matmul fp8e4 with perf_mode=DoubleRow (2 contraction
subtiles per instruction = 2x bf16 throughput; moving operands stream at
~216 ns per 512x(K=256) instruction when warm):

  x   [c, n]    4 tiles [128, 1024] bf16 (halves the HBM-bound input DMA)
  GN: per-channel mean/var (DVE bn_stats for 3 tiles, ACT accum for the
      tile whose DMA lands alongside) into one blocked [128, 12] stats
      tile; ONE gather matmul -> [8, 12] group stats; short batched
      [8,4]/[128,4] chain; ONE scatter matmul -> per-channel a', b'
      (carrying the fp8 scale S_h=16); h = a'x - b'.
  T  [c2, n] = M~^T h    (DR pairs over c1-tiles)        -> fp8, scale 8
  V2 [m, o]  = h^T W2~   (h stationary, DR pairs c-tiles) -> fp8, scale 16
  ST [m, n]  = h^T T     (DR pairs c2-tiles)
  E = exp(ST/(sqrt(C)*S_h*S_T) - 2.5)  (shift cancels in softmax; keeps
      exp() inside fp8e4's 240 max)                       -> fp8, scale 1
  den[n] = (16*ones)^T E  (DR; the 16 bakes in 1/S_V2)
  R = 1/den  (DVE reciprocal; sample 1's reciprocals are emitted after
      sample 0's attnV evacuations so they never stall the PSUM ring)
  O  [o, n]  = V2^T E    (DR pairs m-tiles)  == unnormalized o2
  y = (O*R + c0) + x     (c0 = wo bv + bo host-folded; when c0 == 0 the
      final op is a plain DVE add and y lands bf16 in place over x)

Each phase's two 512-wide PSUM banks are allocated as one [128, 1024]
tile so every evacuation / exp / final op and output DMA covers 1024
columns in a single instruction.  PE warmup runs off a memset tile (no
DMA dependency) and is long enough to hold the HAM at full clock through
the GroupNorm window.  Sample tiles stripe across the HWDGE (sync) and
SWDGE (gpsimd) DMA queues with constants/weights queued behind them;
output DMAs alternate queues.

Softmax-constant terms of the q/k biases cancel exactly; the surviving
term (wk^T bq)^T h is emitted as tiny extra matmuls only when bq/bk are
nonzero (the graph is built per bias-structure and cached).  All fp8
scales are powers of two folded into existing activation scales.

Measured: HW exec ~100.8 us on 8 cores (baseline 221 us), rel err 5.1e-3
(CoreSim-validated; gate 2e-2).
"""

import numpy as np

import concourse.bass as bass
import concourse.mybir as mybir
from concourse import tile
from concourse.bass_utils import run_bass_kernel_spmd


def _install_drain_patch():
    """This walrus build rejects Drain instructions carrying more than one
    semaphore wait (setupSyncWait<CTRL_NO_STRUCT>). Split the TileContext
    tail drain's waits across a chain of single-wait drains."""
    import concourse.tile as tile_mod
    from concourse.vector_clock import ScopedClock

    if getattr(tile_mod.TileContext, "_drain_patch_installed", False):
        return

    def _patched(self, tick_clock, wait_clock):
        nc = self.nc
        drain_inst = nc.sync.drain()
        wait_clock.add_sem_waits(
            drain_inst.ins, ScopedClock({None: tick_clock.global_clock})
        )
        si = drain_inst.ins.sync_info
        waits = list(si.on_wait or []) if si is not None else []
        if len(waits) > 1:
            si.on_wait = waits[:1]
            for w in waits[1:]:
                extra = nc.sync.drain()
                extra.ins.sync_info = mybir.SyncInfo(on_wait=[w], on_update=[])

        nc.all_engine_barrier()
        assert self.sems is not None
        popped = nc._tile_sem_poison_stack.pop()
        assert popped is self._sem_poison
        nc.clear_and_free_semaphores(list(self.sems.allocated().values()))
        nc.all_engine_barrier()

    tile_mod.TileContext._drain_and_barrier = _patched
    tile_mod.TileContext._drain_patch_installed = True


_install_drain_patch()

F32 = mybir.dt.float32
BF16 = mybir.dt.bfloat16
FP8 = mybir.dt.float8e4
DR = mybir.MatmulPerfMode.DoubleRow

B, C, H, W = 16, 512, 32, 32
N = H * W                      # 1024 pixels
NCORES = 8
S = B // NCORES                # samples per core
CT = C // 128                  # 4 channel tiles
NW = 512                       # psum bank width (fp32)
NCH = N // NW                  # 2 chunks
MT = N // 128                  # 8 pixel tiles
GROUPS = 32
GSIZE = C // GROUPS            # 16 channels per group
GPT = 128 // GSIZE             # 8 groups per channel tile
EPS = 1e-5

# fp8 scale plan (all powers of two; folded into existing scalars)
S_H = 16.0                     # h
S_M = 256.0                    # M~ = wq^T wk
S_T = 8.0                      # T
S_W2 = 256.0                   # W2~ = wo wv
S_V2 = 16.0                    # V2 (also baked into the den "ones")
EK = 2.5                        # exp shift, cancels in softmax
T_EVAC = S_T / (S_H * S_M)             # 2^-9
V2_EVAC = S_V2 / (S_H * S_W2)          # 2^-8
E_SCALE = 1.0 / (S_H * S_T * float(np.sqrt(C)))


_MULTIWAIT_OK = (
    mybir.InstTensorTensor, mybir.InstTensorScalarPtr, mybir.InstActivation,
    mybir.InstReciprocal, mybir.InstTensorCopy, mybir.InstMemset,
)


def _split_waits(nc, maxw=1, maxw_elem=1):
    """This walrus build caps the number of sync waits an instruction can
    carry (Drain and Matmult/LDWEIGHTS observed failing with >1). Hoist
    excess waits onto standalone EventSemaphore instructions inserted just
    before, on the same engine. Elementwise instructions tolerate more
    waits, so they keep up to `maxw_elem` and need fewer splits."""
    cnt = 0
    for f in nc.m.functions:
        for bb in f.blocks:
            insts = list(bb.instructions)
            out = []
            changed = False
            for inst in insts:
                si = inst.sync_info
                waits = list(si.on_wait) if (si is not None and si.on_wait) else []
                lim = maxw_elem if isinstance(inst, _MULTIWAIT_OK) else maxw
                if len(waits) > lim:
                    for w in waits[:-lim]:
                        ev = mybir.InstEventSemaphore(
                            name=f"waitsplit_{cnt}", ins=[], outs=[])
                        cnt += 1
                        ev.engine = inst.engine
                        ev.sync_info = mybir.SyncInfo(on_wait=[w], on_update=[])
                        out.append(ev)
                    si.on_wait = waits[-lim:]
                    changed = True
                out.append(inst)
            if changed:
                _replace_block_instructions(bb, out)
    return cnt


def _replace_block_instructions(bb, insts):
    try:
        bb.instructions = insts
        return
    except Exception:
        pass
    try:
        bb.instructions.clear()
        for i in insts:
            bb.instructions.append(i)
        return
    except Exception:
        pass
    raise RuntimeError("cannot rewrite block instructions")


def build_nc(has_qk_bias=False, has_c0=True, split_waits=True):
    nc = bass.Bass(target_bir_lowering=False)

    x_ext = nc.declare_dram_parameter("x", [S, CT, 128, N], BF16, isOutput=False)
    mfus_ext = nc.declare_dram_parameter("mfus", [128, CT, C], FP8, isOutput=False)
    w2fus_ext = nc.declare_dram_parameter("w2fus", [128, CT, C], FP8,
                                          isOutput=False)
    # cblob columns: c0[4] gnw'[4] gnb'[4] gmat[8] -> [128, 20] f32
    cblob_ext = nc.declare_dram_parameter("cblob", [128, 20], F32,
                                          isOutput=False)
    gmt_ext = nc.declare_dram_parameter("gmt", [GPT, 128], F32, isOutput=False)
    rvec_ext = None
    if has_qk_bias:
        rvec_ext = nc.declare_dram_parameter("rvec", [128, CT, 1], FP8,
                                             isOutput=False)
    out_ext = nc.declare_dram_parameter("out", [S, CT, 128, N], BF16,
                                        isOutput=True)

    with tile.TileContext(nc) as tc:
        _body(nc, tc, x_ext, mfus_ext, w2fus_ext, cblob_ext,
              gmt_ext, rvec_ext, out_ext, has_c0)
    if split_waits:
        _split_waits(nc)
    return nc


def _body(nc, tc, x_ext, mfus_ext, w2fus_ext, cblob_ext,
          gmt_ext, rvec_ext, out_ext, has_c0=True):
    import contextlib

    ctx = contextlib.ExitStack()
    with ctx:
        consts = ctx.enter_context(tc.tile_pool(name="consts", bufs=1))
        sb = ctx.enter_context(tc.tile_pool(name="sb", bufs=1))
        ps = ctx.enter_context(tc.tile_pool(name="ps", space="PSUM", bufs=1))

        # ---------------- constants ----------------
        mfus = consts.tile([128, CT, C], FP8, tag="mfus")
        w2fus = consts.tile([128, CT, C], FP8, tag="w2fus")
        cblob = consts.tile([128, 20], F32, tag="cblob")
        gmt = consts.tile([GPT, 128], F32, tag="gmt")

        # den "ones" (value S_V2) from memset: no DMA dependency, and the
        # warmup matmuls can start immediately.
        onesden = consts.tile([128, 2, 128], FP8, tag="onesden")
        nc.vector.memset(onesden, S_V2)

        b_sb = {}
        for bi, b in enumerate(("c0", "gnw", "gnb")):
            b_sb[b] = [cblob[:, bi * CT + ct:bi * CT + ct + 1]
                       for ct in range(CT)]
        gnw4 = cblob[:, 4:8]
        gnb4 = cblob[:, 8:12]
        gmat = cblob[:, 12:12 + GPT]

        rvec = None
        if rvec_ext is not None:
            rvec = consts.tile([128, CT, 1], FP8, tag="rvec")
            nc.gpsimd.dma_start(out=rvec, in_=rvec_ext[:, :, :])

        eps_g = consts.tile([GPT, 1], F32, tag="eps_g")
        nc.vector.memset(eps_g, EPS)
        nek = consts.tile([128, 1], F32, tag="nek")
        nc.vector.memset(nek, -EK)

        # PE warmup off the memset tile: first thing in the PE stream.
        warm = ps.tile([128, NW], F32, tag="small", bufs=2)
        for wi in range(60):
            nc.tensor.matmul(warm[:, 0:128], onesden[:, 0, :], onesden[:, 0, :],
                             start=(wi == 0), stop=(wi == 59))

        def phase_weights():
            # Constants + weights ride the gpsimd SWDGE queue BEHIND the x
            # tiles: x owns the HBM bandwidth window at the head.
            nc.gpsimd.dma_start(out=cblob, in_=cblob_ext[:, :])
            nc.gpsimd.dma_start(out=gmt, in_=gmt_ext[:, :])
            nc.gpsimd.dma_start(out=mfus[:, :, :], in_=mfus_ext[:, :, :])
            nc.gpsimd.dma_start(out=w2fus[:, :, :], in_=w2fus_ext[:, :, :])

        # ---------------- per-sample pipelines, emitted phase-major ----------------
        st = [dict() for _ in range(S)]

        def phase_load(s):
            # Stripe each sample's tiles across both DMA queues (ct0/1 on
            # the HWDGE sync queue, ct2/3 on the SWDGE gpsimd queue) so the
            # tiles of the sample being normalized land pairwise-parallel.
            x_sb = []
            for ct in range(CT):
                xt = sb.tile([128, N], BF16, name=f"x{s}_{ct}", tag=f"x_{ct}",
                             bufs=2)
                eng = nc.sync if ct < 2 else nc.gpsimd
                eng.dma_start(out=xt, in_=x_ext[s, ct, :, :])
                x_sb.append(xt)
            st[s]["x"] = x_sb

        def phase_gn(s):
            x_sb = st[s]["x"]
            # Blocked stats [128, 12] = [mean(4) | q(4) | m2(4)] where
            # q = var + mean^2 on the DVE path, E[x^2] on the ACT path
            # (m2 column zero there) -- downstream uses q + m2 either way.
            stats = sb.tile([128, 12], F32, tag="stats", bufs=2)
            nc.vector.memset(stats[:, 10:11], 0.0)
            for ct in range(CT):
                if ct != 2:
                    st6 = sb.tile([128, 2, 6], F32, tag="st6", bufs=4)
                    nc.vector.bn_stats(out=st6[:, 0, :], in_=x_sb[ct][:, 0:512])
                    nc.vector.bn_stats(out=st6[:, 1, :],
                                       in_=x_sb[ct][:, 512:1024])
                    mv = sb.tile([128, 2], F32, tag=f"mv_{ct}", bufs=2)
                    nc.vector.bn_aggr(out=mv, in_=st6)
                    nc.vector.tensor_copy(out=stats[:, ct:ct + 1],
                                          in_=mv[:, 0:1])
                    nc.vector.tensor_mul(out=stats[:, 8 + ct:9 + ct],
                                         in0=mv[:, 0:1], in1=mv[:, 0:1])
                    nc.vector.tensor_copy(out=stats[:, 4 + ct:5 + ct],
                                          in_=mv[:, 1:2])
                else:
                    scr = sb.tile([128, N], FP8, tag="gnscr", bufs=2)
                    nc.scalar.activation(
                        out=scr, in_=x_sb[ct],
                        func=mybir.ActivationFunctionType.Copy,
                        scale=1.0 / N, accum_out=stats[:, ct:ct + 1])
                    nc.scalar.activation(
                        out=scr, in_=x_sb[ct],
                        func=mybir.ActivationFunctionType.Square,
                        scale=1.0 / float(np.sqrt(N)),
                        accum_out=stats[:, 4 + ct:5 + ct])

            # ONE gather matmul: group stats [8, 12]
            gp = ps.tile([GPT, 12], F32, tag="small", bufs=2)
            nc.tensor.matmul(gp, gmat, stats, start=True, stop=True)
            gs = sb.tile([GPT, 12], F32, tag="gs", bufs=2)
            nc.vector.tensor_copy(out=gs, in_=gp)
            # var_g = (E[q] + E[m2]) - E[mean]^2, batched over the 4 tiles
            m2 = sb.tile([GPT, 2, 4], F32, tag="m2", bufs=2)
            nc.vector.tensor_add(out=m2[:, 0, :], in0=gs[:, 4:8],
                                 in1=gs[:, 8:12])
            nc.vector.tensor_mul(out=m2[:, 1, :], in0=gs[:, 0:4],
                                 in1=gs[:, 0:4])
            s2 = sb.tile([GPT, 2, 4], F32, tag="s2", bufs=2)
            nc.vector.tensor_sub(out=s2[:, 1, :], in0=m2[:, 0, :],
                                 in1=m2[:, 1, :])
            nc.scalar.activation(out=s2[:, 1, :], in_=s2[:, 1, :],
                                 func=mybir.ActivationFunctionType.Sqrt,
                                 bias=eps_g, scale=1.0)
            nc.vector.reciprocal(out=s2[:, 1, :], in_=s2[:, 1, :])
            nc.vector.tensor_copy(out=s2[:, 0, :], in_=gs[:, 0:4])
            # ONE scatter matmul: abp [128, 8] = [mu(4) | 1/sigma(4)]
            abp = ps.tile([128, 2, 4], F32, tag="small", bufs=2)
            nc.tensor.matmul(abp, gmt, s2, start=True, stop=True)
            a4 = sb.tile([128, 4], F32, tag="a4", bufs=2)
            nc.vector.tensor_mul(out=a4, in0=abp[:, 1, :], in1=gnw4)
            bneg4 = sb.tile([128, 4], F32, tag="bneg4", bufs=2)
            nc.vector.tensor_mul(out=bneg4, in0=abp[:, 0, :], in1=a4)
            nc.vector.tensor_sub(out=bneg4, in0=bneg4, in1=gnb4)
            nbneg4 = sb.tile([128, 4], F32, tag="nbneg4", bufs=2)
            nc.vector.tensor_scalar_mul(out=nbneg4, in0=bneg4, scalar1=-1.0)

            ht = sb.tile([128, CT, N], FP8, name=f"h{s}", tag="h", bufs=2)
            for ct in range(CT):
                if ct < 2:
                    nc.vector.tensor_scalar(
                        out=ht[:, ct, :], in0=x_sb[ct],
                        scalar1=a4[:, ct:ct + 1], scalar2=bneg4[:, ct:ct + 1],
                        op0=mybir.AluOpType.mult,
                        op1=mybir.AluOpType.subtract,
                    )
                else:
                    # Identity takes per-partition scale+bias APs and lives
                    # in every activation table.
                    nc.scalar.activation(
                        out=ht[:, ct, :], in_=x_sb[ct],
                        func=mybir.ActivationFunctionType.Identity,
                        scale=a4[:, ct:ct + 1], bias=nbneg4[:, ct:ct + 1])
            st[s]["h"] = ht

        def phase_tv(s):
            ht = st[s]["h"]
            # T[c2-slice, n] = sum_{c1-pairs} M~[:, pair, c2-slice].T @ h
            tt = sb.tile([128, CT, N], FP8, name=f"t{s}", tag="t", bufs=2)
            for ot in range(CT):
                pp = ps.tile([128, N], F32, tag="mm", bufs=3)
                for nch in range(NCH):
                    for cp in range(CT // 2):
                        nc.tensor.matmul(
                            pp[:, nch * NW:(nch + 1) * NW],
                            mfus[:, 2 * cp:2 * cp + 2, ot * 128:(ot + 1) * 128],
                            ht[:, 2 * cp:2 * cp + 2, nch * NW:(nch + 1) * NW],
                            start=(cp == 0), stop=(cp == CT // 2 - 1),
                            perf_mode=DR)
                nc.scalar.activation(
                    out=tt[:, ot, :], in_=pp,
                    func=mybir.ActivationFunctionType.Copy, scale=T_EVAC)
            st[s]["t"] = tt
            # V2[m-slice, o] = sum_{c-pairs} h[:, pair, m-slice].T @ W2~
            v2 = sb.tile([128, MT, C], FP8, name=f"v2{s}", tag="v2", bufs=2)
            for mp in range(MT // 2):
                vp = ps.tile([128, N], F32, tag="mm", bufs=3)
                for half in range(2):
                    mt = 2 * mp + half
                    for cp in range(CT // 2):
                        nc.tensor.matmul(
                            vp[:, half * NW:(half + 1) * NW],
                            ht[:, 2 * cp:2 * cp + 2, mt * 128:(mt + 1) * 128],
                            w2fus[:, 2 * cp:2 * cp + 2, :],
                            start=(cp == 0), stop=(cp == CT // 2 - 1),
                            perf_mode=DR)
                if mp % 2 == 0:
                    nc.vector.tensor_scalar_mul(
                        out=v2[:, 2 * mp:2 * mp + 2, :], in0=vp,
                        scalar1=V2_EVAC)
                else:
                    nc.scalar.activation(
                        out=v2[:, 2 * mp:2 * mp + 2, :], in_=vp,
                        func=mybir.ActivationFunctionType.Copy,
                        scale=V2_EVAC)
            st[s]["v2"] = v2
            # optional q/k-bias softmax term: wvec[m] = (S_h h)^T rvec
            if rvec is not None:
                ebias = sb.tile([128, MT], F32, name=f"eb{s}", tag="ebias",
                                bufs=2)
                for mt in range(MT):
                    wp = ps.tile([128, 1], F32, tag="small", bufs=2)
                    for cp in range(CT // 2):
                        nc.tensor.matmul(
                            wp,
                            ht[:, 2 * cp:2 * cp + 2, mt * 128:(mt + 1) * 128],
                            rvec[:, 2 * cp:2 * cp + 2, :],
                            start=(cp == 0), stop=(cp == CT // 2 - 1),
                            perf_mode=DR)
                    nc.vector.tensor_scalar(
                        out=ebias[:, mt:mt + 1], in0=wp,
                        scalar1=1.0 / (S_H * 256.0 * float(np.sqrt(C))),
                        scalar2=-EK,
                        op0=mybir.AluOpType.mult, op1=mybir.AluOpType.add)
                st[s]["ebias"] = ebias

        def st_alloc(s):
            st[s]["e"] = sb.tile([128, MT, N], FP8, name=f"e{s}", tag="e",
                                 bufs=2)

        def emit_st_group(s, mt):
            ht, tt, et = st[s]["h"], st[s]["t"], st[s]["e"]
            eb = st[s].get("ebias")
            sp = ps.tile([128, N], F32, tag="mm", bufs=3)
            for nch in range(NCH):
                for cp in range(CT // 2):
                    nc.tensor.matmul(
                        sp[:, nch * NW:(nch + 1) * NW],
                        ht[:, 2 * cp:2 * cp + 2, mt * 128:(mt + 1) * 128],
                        tt[:, 2 * cp:2 * cp + 2, nch * NW:(nch + 1) * NW],
                        start=(cp == 0), stop=(cp == CT // 2 - 1),
                        perf_mode=DR)
            nc.scalar.activation(
                out=et[:, mt, :], in_=sp,
                func=mybir.ActivationFunctionType.Exp,
                scale=E_SCALE,
                bias=(eb[:, mt:mt + 1] if eb is not None else nek),
            )

        def phase_st(s):
            st_alloc(s)
            for mt in range(MT):
                emit_st_group(s, mt)

        def phase_den_mm(s):
            et = st[s]["e"]
            dps = []
            for nch in range(NCH):
                dp = ps.tile([128, NW], F32, tag="small", bufs=2)
                for mp in range(MT // 2):
                    nc.tensor.matmul(
                        dp, onesden,
                        et[:, 2 * mp:2 * mp + 2, nch * NW:(nch + 1) * NW],
                        start=(mp == 0), stop=(mp == MT // 2 - 1),
                        perf_mode=DR)
                dps.append(dp)
            st[s]["dps"] = dps

        def phase_den_recip(s):
            # Emitted separately so sample 1's reciprocals queue on DVE
            # AFTER sample 0's attnV evacuations (the den PSUMs just wait).
            R_sb = sb.tile([128, N], F32, name=f"R{s}", tag="R", bufs=2)
            for nch in range(NCH):
                # R = 1/(S_V2 * S_E * den): the S_V2 rides the ones value
                nc.vector.reciprocal(out=R_sb[:, nch * NW:(nch + 1) * NW],
                                     in_=st[s]["dps"][nch])
            st[s]["R"] = R_sb

        def emit_av_group(s, ot):
            x_sb, et, v2, R_sb = st[s]["x"], st[s]["e"], st[s]["v2"], st[s]["R"]
            op_ = ps.tile([128, N], F32, tag="mm", bufs=3)
            for nch in range(NCH):
                for mp in range(MT // 2):
                    nc.tensor.matmul(
                        op_[:, nch * NW:(nch + 1) * NW],
                        v2[:, 2 * mp:2 * mp + 2, ot * 128:(ot + 1) * 128],
                        et[:, 2 * mp:2 * mp + 2, nch * NW:(nch + 1) * NW],
                        start=(mp == 0), stop=(mp == MT // 2 - 1),
                        perf_mode=DR)
            # bf16 tmp: the residual add below is then all-2-byte, eligible
            # for the DVE 2x mode (O is ~0.04-scale, bf16 noise negligible)
            tmp = sb.tile([128, N], BF16, tag="tmp", bufs=4)
            nc.vector.tensor_mul(out=tmp, in0=op_, in1=R_sb)
            if has_c0:
                # y = (O + c0) + x, written in place over x
                nc.vector.scalar_tensor_tensor(
                    out=x_sb[ot], in0=tmp, scalar=b_sb["c0"][ot],
                    in1=x_sb[ot],
                    op0=mybir.AluOpType.add, op1=mybir.AluOpType.add,
                )
            else:
                # c0 == 0: plain residual add (y lands bf16 in place)
                nc.vector.tensor_add(out=x_sb[ot], in0=tmp, in1=x_sb[ot])
            eng = nc.sync if ot % 2 == 0 else nc.gpsimd
            eng.dma_start(out=out_ext[s, ot, :, :], in_=x_sb[ot])

        # x(s0) first (feeds GroupNorm), then x(s1); weights on the other
        # queue. den/recip(s) rides right behind st(s) so R(s) is ready
        # long before phase_av(s) needs it.
        phase_load(0)
        phase_load(1)
        phase_weights()
        for s in range(S):
            phase_gn(s)
        for s in range(S):
            phase_tv(s)
        phase_st(0)
        phase_den_mm(0)
        phase_den_recip(0)
        # sample 1's score groups interleave with sample 0's attnV groups:
        # av's PSUM evacuations drain while the PE chews ST matmuls.
        st_alloc(1)
        for mt in range(MT):
            emit_st_group(1, mt)
            if mt % 2 == 0:
                emit_av_group(0, mt // 2)
        phase_den_mm(1)
        phase_den_recip(1)
        for ot in range(CT):
            emit_av_group(1, ot)


_CACHE = {}


def _q8(v, scale):
    import ml_dtypes
    return np.clip(np.asarray(v, np.float32) * scale, -240.0, 240.0).astype(
        ml_dtypes.float8_e4m3)


def make_in_maps(inputs):
    """Host-side weight folding + layout prep shared by kernel() and the
    test/sim harnesses. Returns (in_maps, has_qk_bias)."""
    x = np.asarray(inputs["x"], dtype=np.float32)
    assert x.shape == (B, C, H, W)

    wq = np.asarray(inputs["wq"], np.float64)
    wk = np.asarray(inputs["wk"], np.float64)
    wv = np.asarray(inputs["wv"], np.float64)
    wo = np.asarray(inputs["wo"], np.float64)
    bq = np.asarray(inputs["bq"], np.float64)
    bk = np.asarray(inputs["bk"], np.float64)

    # scores = h^T M h with M[c1,c2];  T[c2,n] = sum_c1 M[c1,c2] h[c1,n]
    M = wq.T @ wk
    # V2[m,o] = sum_c W2[o,c] h[c,m];  moving operand W2T[c,o]
    W2T = (wo @ wv).T
    mfus = np.ascontiguousarray(
        M.reshape(CT, 128, C).transpose(1, 0, 2))       # [128, ct(c1), c2]
    w2fus = np.ascontiguousarray(
        W2T.reshape(CT, 128, C).transpose(1, 0, 2))     # [128, ct(c), o]

    c0 = (wo @ np.asarray(inputs["bv"], np.float64)
          + np.asarray(inputs["bo"], np.float64)).astype(np.float32)

    gmat = np.zeros((128, GPT), dtype=np.float32)
    gmt = np.zeros((GPT, 128), dtype=np.float32)
    for g in range(GPT):
        gmat[g * GSIZE:(g + 1) * GSIZE, g] = 1.0 / GSIZE
        gmt[g, g * GSIZE:(g + 1) * GSIZE] = 1.0

    cblob = np.zeros((128, 20), dtype=np.float32)
    gnw = np.asarray(inputs["gn_weight"], np.float32) * S_H
    gnb = np.asarray(inputs["gn_bias"], np.float32) * S_H
    for bi, arr in enumerate((c0, gnw, gnb)):
        cblob[:, bi * CT:(bi + 1) * CT] = np.asarray(
            arr, dtype=np.float32).reshape(CT, 128).T
    cblob[:, 12:12 + GPT] = gmat

    base = {
        "mfus": _q8(mfus, S_M),
        "w2fus": _q8(w2fus, S_W2),
        "cblob": cblob,
        "gmt": gmt,
    }

    has_qk_bias = bool(np.any(bq) or np.any(bk))
    if has_qk_bias:
        rv = (wk.T @ bq)                          # [C]; scale S_r = 256
        base["rvec"] = _q8(rv.reshape(CT, 128).T.reshape(128, CT, 1), 256.0)

    import ml_dtypes
    xr = x.reshape(NCORES, S, CT, 128, N).astype(ml_dtypes.bfloat16)
    return ([dict(base, x=np.ascontiguousarray(xr[i])) for i in range(NCORES)],
            has_qk_bias, bool(np.any(c0)))


def kernel(**inputs):
    in_maps, has_qk_bias, has_c0 = make_in_maps(inputs)
    key = ("nc", has_qk_bias, has_c0)
    if key not in _CACHE:
        _CACHE[key] = build_nc(has_qk_bias=has_qk_bias, has_c0=has_c0)
    nc = _CACHE[key]

    res = run_bass_kernel_spmd(nc, in_maps, core_ids=list(range(NCORES)))

    out = np.empty((NCORES, S, CT, 128, N), dtype=np.float32)
    for i in range(NCORES):
        out[i] = np.asarray(res.results[i]["out"], dtype=np.float32)
    return out.reshape(B, C, H, W)
